# revision 10
# baseline (speedup 1.0000x reference)
"""Multi-head attention (B=4, S=1024, H=1024, heads=16) on 8 trn2 NeuronCores.

Sharding: data-parallel over batch (4) x tensor-parallel over head-groups (2).
Core c handles batch c//2, heads [8*(c%2), 8*(c%2)+8).

Per-core kernel (all matmuls bf16 with fp32 PSUM accumulation):
  - projections: qh_T/kh_T in [d, i] layout (head dim on partitions), vh in
    [j, hd] layout augmented with a ones column per head.
  - scores computed transposed (keys on partitions): s_T = kh_T^T-chunks @ qh_T,
    exp on ScalarE, multiplied by host-precomputed exp(attn_bias)^T.
  - ctx_T and the softmax denominator come from ONE matmul per (head, i-chunk):
    stationary = [vh | ones] (65 cols), accumulated over key blocks.
  - normalize with reciprocal + gpsimd partition broadcast, output projection
    with row-parallel Wo; host adds the two partial results + bo.

Scale (1/8) is folded into Wq/bq on the host. Softmax max-subtraction is
skipped: scores+bias are within +-8 so exp is well-conditioned in fp32.
"""

import numpy as np
import ml_dtypes

BF16 = ml_dtypes.bfloat16

S = 1024
HID = 1024
GCOL = 512  # hidden cols per core (8 heads * 64)
DH = 64
P = 128
NPAIR = 4  # head pairs per core
NJB = 8  # key blocks of 128
NCB = 8  # contraction blocks of 128
NIB = 8  # query blocks of 128

_CACHED_NC = None
DEBUG_TAPS = False


def _build_nc():
    import concourse.bass as bass
    import concourse.mybir as mybir
    import concourse.tile as tile
    from concourse import bacc
    from contextlib import ExitStack

    f32 = mybir.dt.float32
    bf16 = mybir.dt.bfloat16
    AF = mybir.ActivationFunctionType

    nc = bacc.Bacc(
        "TRN2",
        target_bir_lowering=False,
        debug=False,
        enable_asserts=False,
        num_devices=8,
    )

    qT = nc.dram_tensor("qT", [HID, S], bf16, kind="ExternalInput").ap()
    kT = nc.dram_tensor("kT", [HID, S], bf16, kind="ExternalInput").ap()
    vT = nc.dram_tensor("vT", [HID, S], bf16, kind="ExternalInput").ap()
    wq = nc.dram_tensor("wq", [HID, GCOL], bf16, kind="ExternalInput").ap()
    wk = nc.dram_tensor("wk", [HID, GCOL], bf16, kind="ExternalInput").ap()
    wv = nc.dram_tensor("wv", [HID, GCOL], bf16, kind="ExternalInput").ap()
    wo = nc.dram_tensor("wo", [GCOL, HID], bf16, kind="ExternalInput").ap()
    bq = nc.dram_tensor("bq", [GCOL], f32, kind="ExternalInput").ap()
    bk = nc.dram_tensor("bk", [GCOL], f32, kind="ExternalInput").ap()
    bv = nc.dram_tensor("bv", [GCOL], bf16, kind="ExternalInput").ap()
    expb = nc.dram_tensor("expb", [8, S, S], bf16, kind="ExternalInput").ap()
    out = nc.dram_tensor("out", [S, HID], f32, kind="ExternalOutput").ap()
    if DEBUG_TAPS:
        dbg_qhT = nc.dram_tensor("dbg_qhT", [NPAIR, P, S], bf16, kind="ExternalOutput").ap()
        dbg_khT = nc.dram_tensor("dbg_khT", [NPAIR, P, S], bf16, kind="ExternalOutput").ap()
        dbg_vh = nc.dram_tensor("dbg_vh", [NJB, P, 8, DH + 1], bf16, kind="ExternalOutput").ap()
        dbg_e = nc.dram_tensor("dbg_e", [2, NJB, P, S], bf16, kind="ExternalOutput").ap()
        dbg_ctxn = nc.dram_tensor("dbg_ctxn", [NPAIR, P, S], bf16, kind="ExternalOutput").ap()
        dbg_cr = nc.dram_tensor("dbg_cr", [2, 2, DH + 1, 512], f32, kind="ExternalOutput").ap()
        dbg_rbc = nc.dram_tensor("dbg_rbc", [2, 2, DH + 1, 512], f32, kind="ExternalOutput").ap()

    with tile.TileContext(nc) as tc, ExitStack() as ctx:
        const = ctx.enter_context(tc.tile_pool(name="const", bufs=1))
        inT = ctx.enter_context(tc.tile_pool(name="inT", bufs=16))
        proj = ctx.enter_context(tc.tile_pool(name="proj", bufs=1))
        work = ctx.enter_context(tc.tile_pool(name="work", bufs=4))
        outp = ctx.enter_context(tc.tile_pool(name="outp", bufs=3))
        psum = ctx.enter_context(tc.tile_pool(name="psum", bufs=2, space="PSUM"))

        # ---- constants / weights ----
        wq_sb = const.tile([P, NCB, GCOL], bf16, tag="wq")
        wk_sb = const.tile([P, NCB, GCOL], bf16, tag="wk")
        wv_sb = const.tile([P, NCB, GCOL], bf16, tag="wv")
        wo_sb = const.tile([P, NPAIR, HID], bf16, tag="wo")
        nc.sync.dma_start(out=wq_sb, in_=wq.rearrange("(cb p) n -> p cb n", p=P))
        nc.sync.dma_start(out=wk_sb, in_=wk.rearrange("(cb p) n -> p cb n", p=P))
        nc.sync.dma_start(out=wv_sb, in_=wv.rearrange("(cb p) n -> p cb n", p=P))
        nc.sync.dma_start(out=wo_sb, in_=wo.rearrange("(pr p) n -> p pr n", p=P))
        bq_sb = const.tile([P, NPAIR], f32, tag="bq")
        bk_sb = const.tile([P, NPAIR], f32, tag="bk")
        nc.sync.dma_start(out=bq_sb, in_=bq.rearrange("(pr p) -> p pr", p=P))
        nc.sync.dma_start(out=bk_sb, in_=bk.rearrange("(pr p) -> p pr", p=P))
        bv_sb = const.tile([1, GCOL], bf16, tag="bv")
        nc.sync.dma_start(out=bv_sb, in_=bv.rearrange("(a n) -> a n", a=1))
        ones_k1 = const.tile([1, P], bf16, tag="ones_k1")
        nc.vector.memset(ones_k1, 1.0)

        qhT = [proj.tile([P, S], bf16, name=f"qhT{i}", tag=f"qhT{i}") for i in range(NPAIR)]
        khT = [proj.tile([P, S], bf16, name=f"khT{i}", tag=f"khT{i}") for i in range(NPAIR)]
        # vh_sb[jb]: [j in block, head, 65] where col 64 is ones (denominator trick)
        vh_sb = [proj.tile([P, 8, DH + 1], bf16, name=f"vh{i}", tag=f"vh{i}") for i in range(NJB)]
        ctxn = [proj.tile([P, S], bf16, name=f"ctxn{i}", tag=f"ctxn{i}") for i in range(NPAIR)]

        # ---- projections: q, k -> [d, i] transposed head layout ----
        for tname, src, w_sb, b_sb, dst in (
            ("q", qT, wq_sb, bq_sb, qhT),
            ("k", kT, wk_sb, bk_sb, khT),
        ):
            tiles = []
            for cb in range(NCB):
                t = inT.tile([P, S], bf16, name=f"{tname}T{cb}", tag="inT")
                nc.sync.dma_start(out=t, in_=src[cb * P:(cb + 1) * P, :])
                tiles.append(t)
            for pr in range(NPAIR):
                ps = psum.tile([P, S], f32, tag="mm")
                for ic in range(2):
                    for cb in range(NCB):
                        nc.tensor.matmul(
                            ps[:, ic * 512:(ic + 1) * 512],
                            lhsT=w_sb[:, cb, pr * P:(pr + 1) * P],
                            rhs=tiles[cb][:, ic * 512:(ic + 1) * 512],
                            start=(cb == 0),
                            stop=(cb == NCB - 1),
                        )
                nc.vector.tensor_scalar_add(dst[pr], ps, b_sb[:, pr:pr + 1])

        # ---- projection: v -> [j, head, d] with ones column ----
        vtiles = []
        for cb in range(NCB):
            t = inT.tile([P, S], bf16, name=f"vT{cb}", tag="inT")
            nc.sync.dma_start(out=t, in_=vT[cb * P:(cb + 1) * P, :])
            vtiles.append(t)
        for jb in range(NJB):
            ps = psum.tile([P, GCOL], f32, tag="mm")
            for cb in range(NCB):
                nc.tensor.matmul(
                    ps,
                    lhsT=vtiles[cb][:, jb * P:(jb + 1) * P],
                    rhs=wv_sb[:, cb, :],
                    start=(cb == 0),
                    stop=False,
                )
            nc.tensor.matmul(ps, lhsT=ones_k1, rhs=bv_sb, start=False, stop=True)
            nc.vector.tensor_copy(
                out=vh_sb[jb][:, :, 0:DH],
                in_=ps.rearrange("p (h d) -> p h d", d=DH),
            )
            nc.vector.memset(vh_sb[jb][:, :, DH:DH + 1], 1.0)

        # ---- attention per head pair ----
        for pr in range(NPAIR):
            cr = {}
            for hl in range(2):
                for ic in range(2):
                    cr[(hl, ic)] = psum.tile(
                        [DH + 1, 512], f32, name=f"cr{pr}_{hl}_{ic}", tag="cr", bufs=4
                    )
            for jb in range(NJB):
                for hl in range(2):
                    h = 2 * pr + hl
                    eb = work.tile([P, S], bf16, name=f"eb{h}_{jb}", tag="eb")
                    nc.sync.dma_start(out=eb, in_=expb[h, jb * P:(jb + 1) * P, :])
                    s_ps = psum.tile([P, S], f32, name=f"s{h}_{jb}", tag="mm")
                    for ic in range(2):
                        nc.tensor.matmul(
                            s_ps[:, ic * 512:(ic + 1) * 512],
                            lhsT=khT[pr][hl * DH:(hl + 1) * DH, jb * P:(jb + 1) * P],
                            rhs=qhT[pr][hl * DH:(hl + 1) * DH, ic * 512:(ic + 1) * 512],
                            start=True,
                            stop=True,
                        )
                    es = work.tile([P, S], bf16, name=f"es{h}_{jb}", tag="es")
                    nc.scalar.activation(es, s_ps, AF.Exp)
                    e = work.tile([P, S], bf16, name=f"e{h}_{jb}", tag="e")
                    nc.vector.tensor_mul(e, es, eb)
                    if DEBUG_TAPS and pr == 0:
                        nc.sync.dma_start(out=dbg_e[hl, jb], in_=e)
                    for ic in range(2):
                        nc.tensor.matmul(
                            cr[(hl, ic)],
                            lhsT=vh_sb[jb][:, h, :],
                            rhs=e[:, ic * 512:(ic + 1) * 512],
                            start=(jb == 0),
                            stop=(jb == NJB - 1),
                        )
            for hl in range(2):
                for ic in range(2):
                    rbc = work.tile([DH + 1, 512], f32, name=f"rbc{hl}_{ic}", tag="rbc")
                    if DEBUG_TAPS and pr == 0:
                        crs = work.tile([DH + 1, 512], f32, name=f"crs{hl}_{ic}", tag="crs")
                        nc.vector.tensor_copy(out=crs, in_=cr[(hl, ic)])
                        nc.sync.dma_start(out=dbg_cr[hl, ic], in_=crs)
                    nc.vector.reciprocal(rbc[DH:DH + 1, :], cr[(hl, ic)][DH:DH + 1, :])
                    row = rbc[DH:DH + 1, :]
                    row_bcast = bass.AP(
                        tensor=row.tensor,
                        offset=row.offset,
                        ap=[list(row.ap[0]), [0, DH]] + [list(d) for d in row.ap[1:]],
                    )
                    nc.sync.dma_start(out=rbc[0:DH, :], in_=row_bcast)
                    if DEBUG_TAPS and pr == 0:
                        nc.sync.dma_start(out=dbg_rbc[hl, ic], in_=rbc)
                    if hl == 0:
                        nc.vector.tensor_mul(
                            ctxn[pr][0:DH, ic * 512:(ic + 1) * 512],
                            cr[(hl, ic)][0:DH, :],
                            rbc[0:DH, :],
                        )
                    else:
                        ch = work.tile([DH, 512], bf16, name=f"ch{hl}_{ic}", tag="ch")
                        nc.vector.tensor_mul(ch, cr[(hl, ic)][0:DH, :], rbc[0:DH, :])
                        nc.sync.dma_start(
                            out=ctxn[pr][DH:2 * DH, ic * 512:(ic + 1) * 512], in_=ch
                        )

        if DEBUG_TAPS:
            for pr in range(NPAIR):
                nc.sync.dma_start(out=dbg_qhT[pr], in_=qhT[pr])
                nc.sync.dma_start(out=dbg_khT[pr], in_=khT[pr])
                nc.sync.dma_start(out=dbg_ctxn[pr], in_=ctxn[pr])
            for jb in range(NJB):
                nc.sync.dma_start(out=dbg_vh[jb], in_=vh_sb[jb])

        # ---- output projection ----
        for ib in range(NIB):
            yp = psum.tile([P, HID], f32, name=f"yp{ib}", tag="mm")
            for cc in range(2):
                for pr in range(NPAIR):
                    nc.tensor.matmul(
                        yp[:, cc * 512:(cc + 1) * 512],
                        lhsT=ctxn[pr][:, ib * P:(ib + 1) * P],
                        rhs=wo_sb[:, pr, cc * 512:(cc + 1) * 512],
                        start=(pr == 0),
                        stop=(pr == NPAIR - 1),
                    )
            y_sb = outp.tile([P, HID], f32, name=f"y{ib}", tag="y")
            nc.scalar.activation(y_sb, yp, AF.Copy)
            nc.sync.dma_start(out=out[ib * P:(ib + 1) * P, :], in_=y_sb)

    nc.compile()
    return nc


def _get_nc():
    global _CACHED_NC
    if _CACHED_NC is None:
        _CACHED_NC = _build_nc()
    return _CACHED_NC


def make_in_maps(q, k, v, attn_bias, Wq, Wk, Wv, Wo, bq, bk, bv, bo):
    scale = DH ** (-0.5)
    in_maps = []
    for core in range(8):
        b, g = divmod(core, 2)
        gs = slice(g * GCOL, (g + 1) * GCOL)
        in_maps.append({
            "qT": np.ascontiguousarray(q[b].T).astype(BF16),
            "kT": np.ascontiguousarray(k[b].T).astype(BF16),
            "vT": np.ascontiguousarray(v[b].T).astype(BF16),
            "wq": (Wq[:, gs] * scale).astype(BF16),
            "wk": np.ascontiguousarray(Wk[:, gs]).astype(BF16),
            "wv": np.ascontiguousarray(Wv[:, gs]).astype(BF16),
            "wo": np.ascontiguousarray(Wo[gs, :]).astype(BF16),
            "bq": (bq[gs] * scale).astype(np.float32),
            "bk": np.ascontiguousarray(bk[gs]).astype(np.float32),
            "bv": np.ascontiguousarray(bv[gs]).astype(BF16),
            "expb": np.exp(
                attn_bias[b, g * 8:(g + 1) * 8].transpose(0, 2, 1)
            ).astype(BF16),
        })
    return in_maps


def kernel(q, k, v, attn_bias, Wq, Wk, Wv, Wo, bq, bk, bv, bo, _trace=False):
    from concourse.bass_utils import run_bass_kernel_spmd

    args = [np.asarray(x, dtype=np.float32) for x in
            (q, k, v, attn_bias, Wq, Wk, Wv, Wo, bq, bk, bv, bo)]
    q, k, v, attn_bias, Wq, Wk, Wv, Wo, bq, bk, bv, bo = args
    nc = _get_nc()
    in_maps = make_in_maps(q, k, v, attn_bias, Wq, Wk, Wv, Wo, bq, bk, bv, bo)
    res = run_bass_kernel_spmd(nc, in_maps, core_ids=list(range(8)), trace=_trace)
    y = np.zeros((4, S, HID), np.float32)
    for core in range(8):
        y[core // 2] += res.results[core]["out"]
    y += bo
    if _trace:
        kernel.last_results = res
    return y


# revision 22
# speedup vs baseline: 1.4124x; 1.4124x over previous
"""Multi-head attention (B=4, S=1024, H=1024, heads=16) on 8 trn2 NeuronCores.

Sharding: data-parallel over batch (4) x tensor-parallel over head-groups (2).
Core c handles batch c//2, heads [8*(c%2), 8*(c%2)+8).

Per-core kernel (all matmuls bf16 with fp32 PSUM accumulation):
  - projections: qh_T/kh_T in [d, i] layout (head dim on partitions), vh in
    [j, hd] layout augmented with a ones column per head.
  - scores computed transposed (keys on partitions): s_T = kh_T^T-chunks @ qh_T,
    exp on ScalarE, multiplied by host-precomputed exp(attn_bias)^T.
  - ctx_T and the softmax denominator come from ONE matmul per (head, i-chunk):
    stationary = [vh | ones] (65 cols), accumulated over key blocks.
  - normalize with reciprocal + gpsimd partition broadcast, output projection
    with row-parallel Wo; host adds the two partial results + bo.

Scale (1/8) is folded into Wq/bq on the host. Softmax max-subtraction is
skipped: scores+bias are within +-8 so exp is well-conditioned in fp32.
"""

import numpy as np
import ml_dtypes

BF16 = ml_dtypes.bfloat16

S = 1024
HID = 1024
GCOL = 512  # hidden cols per core (8 heads * 64)
DH = 64
P = 128
NPAIR = 4  # head pairs per core
NJB = 8  # key blocks of 128
NCB = 8  # contraction blocks of 128
NIB = 8  # query blocks of 128

_CACHED_NC = None
DEBUG_TAPS = False


def _build_nc():
    import concourse.bass as bass
    import concourse.mybir as mybir
    import concourse.tile as tile
    from concourse import bacc
    from contextlib import ExitStack

    f32 = mybir.dt.float32
    bf16 = mybir.dt.bfloat16
    AF = mybir.ActivationFunctionType

    nc = bacc.Bacc(
        "TRN2",
        target_bir_lowering=False,
        debug=False,
        enable_asserts=False,
        num_devices=8,
    )

    qT = nc.dram_tensor("qT", [HID, S], bf16, kind="ExternalInput").ap()
    kT = nc.dram_tensor("kT", [HID, S], bf16, kind="ExternalInput").ap()
    vT = nc.dram_tensor("vT", [HID, S], bf16, kind="ExternalInput").ap()
    wq = nc.dram_tensor("wq", [HID, GCOL], bf16, kind="ExternalInput").ap()
    wk = nc.dram_tensor("wk", [HID, GCOL], bf16, kind="ExternalInput").ap()
    wv = nc.dram_tensor("wv", [HID, GCOL], bf16, kind="ExternalInput").ap()
    wo = nc.dram_tensor("wo", [GCOL, HID], bf16, kind="ExternalInput").ap()
    bq = nc.dram_tensor("bq", [GCOL], f32, kind="ExternalInput").ap()
    bk = nc.dram_tensor("bk", [GCOL], f32, kind="ExternalInput").ap()
    bv = nc.dram_tensor("bv", [GCOL], bf16, kind="ExternalInput").ap()
    expb = nc.dram_tensor("expb", [8, S, S], bf16, kind="ExternalInput").ap()
    out = nc.dram_tensor("out", [S, HID], f32, kind="ExternalOutput").ap()
    if DEBUG_TAPS:
        dbg_qhT = nc.dram_tensor("dbg_qhT", [NPAIR, P, S], bf16, kind="ExternalOutput").ap()
        dbg_khT = nc.dram_tensor("dbg_khT", [NPAIR, P, S], bf16, kind="ExternalOutput").ap()
        dbg_vh = nc.dram_tensor("dbg_vh", [NJB, P, 8, DH + 1], bf16, kind="ExternalOutput").ap()
        dbg_e = nc.dram_tensor("dbg_e", [2, NJB, P, S], bf16, kind="ExternalOutput").ap()
        dbg_ctxn = nc.dram_tensor("dbg_ctxn", [NPAIR, P, S], bf16, kind="ExternalOutput").ap()
        dbg_cr = nc.dram_tensor("dbg_cr", [2, 2, DH + 1, 512], f32, kind="ExternalOutput").ap()
        dbg_rbc = nc.dram_tensor("dbg_rbc", [2, 2, DH + 1, 512], f32, kind="ExternalOutput").ap()

    with tile.TileContext(nc) as tc, ExitStack() as ctx:
        const = ctx.enter_context(tc.tile_pool(name="const", bufs=1))
        inT = ctx.enter_context(tc.tile_pool(name="inT", bufs=16))
        proj = ctx.enter_context(tc.tile_pool(name="proj", bufs=1))
        work = ctx.enter_context(tc.tile_pool(name="work", bufs=6))
        outp = ctx.enter_context(tc.tile_pool(name="outp", bufs=3))
        psum = ctx.enter_context(tc.tile_pool(name="psum", bufs=2, space="PSUM"))

        # ---- constants / weights ----
        wq_sb = const.tile([P, NCB, GCOL], bf16, tag="wq")
        wk_sb = const.tile([P, NCB, GCOL], bf16, tag="wk")
        wv_sb = const.tile([P, NCB, GCOL], bf16, tag="wv")
        wo_sb = const.tile([P, NPAIR, HID], bf16, tag="wo")
        wq_r = wq.rearrange("(cb p) n -> p cb n", p=P)
        wk_r = wk.rearrange("(cb p) n -> p cb n", p=P)
        wv_r = wv.rearrange("(cb p) n -> p cb n", p=P)
        bq_sb = const.tile([P, NPAIR], f32, tag="bq")
        bk_sb = const.tile([P, NPAIR], f32, tag="bk")
        nc.sync.dma_start(out=bq_sb, in_=bq.rearrange("(pr p) -> p pr", p=P))
        nc.sync.dma_start(out=bk_sb, in_=bk.rearrange("(pr p) -> p pr", p=P))
        bv_sb = const.tile([1, GCOL], bf16, tag="bv")
        nc.sync.dma_start(out=bv_sb, in_=bv.rearrange("(a n) -> a n", a=1))
        ones_k1 = const.tile([1, P], bf16, tag="ones_k1")
        nc.vector.memset(ones_k1, 1.0)

        qhT = [proj.tile([P, S], bf16, name=f"qhT{i}", tag=f"qhT{i}") for i in range(NPAIR)]
        khT = [proj.tile([P, S], bf16, name=f"khT{i}", tag=f"khT{i}") for i in range(NPAIR)]
        # vh_sb[jb]: [j in block, head, 65] where col 64 is ones (denominator trick)
        vh_sb = [proj.tile([P, 8, DH + 1], bf16, name=f"vh{i}", tag=f"vh{i}") for i in range(NJB)]
        ctxn = [proj.tile([P, S], bf16, name=f"ctxn{i}", tag=f"ctxn{i}") for i in range(NPAIR)]

        # ---- projections: q, k -> [d, i] transposed head layout ----
        for tname, src, w_sb, w_r, b_sb, dst in (
            ("q", qT, wq_sb, wq_r, bq_sb, qhT),
            ("k", kT, wk_sb, wk_r, bk_sb, khT),
        ):
            tiles = []
            for cb in range(NCB):
                nc.sync.dma_start(out=w_sb[:, cb, :], in_=w_r[:, cb, :])
                t = inT.tile([P, S], bf16, name=f"{tname}T{cb}", tag="inT")
                nc.sync.dma_start(out=t, in_=src[cb * P:(cb + 1) * P, :])
                tiles.append(t)
            for pr in range(NPAIR):
                ps = psum.tile([P, S], f32, tag="mm")
                for cb in range(NCB):
                    for ic in range(2):
                        nc.tensor.matmul(
                            ps[:, ic * 512:(ic + 1) * 512],
                            lhsT=w_sb[:, cb, pr * P:(pr + 1) * P],
                            rhs=tiles[cb][:, ic * 512:(ic + 1) * 512],
                            start=(cb == 0),
                            stop=(cb == NCB - 1),
                        )
                nc.vector.tensor_scalar_add(dst[pr], ps, b_sb[:, pr:pr + 1])

        # ---- projection: v -> [j, head, d] with ones column ----
        vtiles = []
        for cb in range(NCB):
            nc.sync.dma_start(out=wv_sb[:, cb, :], in_=wv_r[:, cb, :])
            t = inT.tile([P, S], bf16, name=f"vT{cb}", tag="inT")
            nc.sync.dma_start(out=t, in_=vT[cb * P:(cb + 1) * P, :])
            vtiles.append(t)
        for jb in range(NJB):
            ps = psum.tile([P, GCOL], f32, tag="mm")
            for cb in range(NCB):
                nc.tensor.matmul(
                    ps,
                    lhsT=vtiles[cb][:, jb * P:(jb + 1) * P],
                    rhs=wv_sb[:, cb, :],
                    start=(cb == 0),
                    stop=False,
                )
            nc.tensor.matmul(ps, lhsT=ones_k1, rhs=bv_sb, start=False, stop=True)
            nc.vector.tensor_copy(
                out=vh_sb[jb][:, :, 0:DH],
                in_=ps.rearrange("p (h d) -> p h d", d=DH),
            )
            nc.vector.memset(vh_sb[jb][:, :, DH:DH + 1], 1.0)

        # wo prefetches during attention
        for pr in range(NPAIR):
            nc.sync.dma_start(
                out=wo_sb[:, pr, :],
                in_=wo.rearrange("(pr p) n -> p pr n", p=P)[:, pr, :],
            )

        # ---- attention per head pair ----
        for pr in range(NPAIR):
            cr = {}
            for hl in range(2):
                for ic in range(2):
                    cr[(hl, ic)] = psum.tile(
                        [DH + 1, 512], f32, name=f"cr{pr}_{hl}_{ic}", tag="cr", bufs=4
                    )
            for jb in range(NJB):
                for hl in range(2):
                    h = 2 * pr + hl
                    eb = work.tile([P, S], bf16, name=f"eb{h}_{jb}", tag="eb")
                    nc.sync.dma_start(out=eb, in_=expb[h, jb * P:(jb + 1) * P, :])
                    s_ps = psum.tile([P, S], f32, name=f"s{h}_{jb}", tag="mm")
                    for ic in range(2):
                        nc.tensor.matmul(
                            s_ps[:, ic * 512:(ic + 1) * 512],
                            lhsT=khT[pr][hl * DH:(hl + 1) * DH, jb * P:(jb + 1) * P],
                            rhs=qhT[pr][hl * DH:(hl + 1) * DH, ic * 512:(ic + 1) * 512],
                            start=True,
                            stop=True,
                        )
                    es = work.tile([P, S], bf16, name=f"es{h}_{jb}", tag="es")
                    nc.scalar.activation(es, s_ps, AF.Exp)
                    e = work.tile([P, S], bf16, name=f"e{h}_{jb}", tag="e")
                    nc.vector.tensor_mul(e, es, eb)
                    if DEBUG_TAPS and pr == 0:
                        nc.sync.dma_start(out=dbg_e[hl, jb], in_=e)
                    for ic in range(2):
                        nc.tensor.matmul(
                            cr[(hl, ic)],
                            lhsT=vh_sb[jb][:, h, :],
                            rhs=e[:, ic * 512:(ic + 1) * 512],
                            start=(jb == 0),
                            stop=(jb == NJB - 1),
                        )
            for hl in range(2):
                for ic in range(2):
                    rbc = work.tile([DH + 1, 512], f32, name=f"rbc{hl}_{ic}", tag="rbc")
                    if DEBUG_TAPS and pr == 0:
                        crs = work.tile([DH + 1, 512], f32, name=f"crs{hl}_{ic}", tag="crs")
                        nc.vector.tensor_copy(out=crs, in_=cr[(hl, ic)])
                        nc.sync.dma_start(out=dbg_cr[hl, ic], in_=crs)
                    # stage raw r (psum row 64) to sbuf, two-stage partition
                    # broadcast of the raw row, then invert the whole block
                    # (custom-DVE recip only works at base partition 0).
                    nc.vector.tensor_copy(rbc[DH:DH + 1, :], cr[(hl, ic)][DH:DH + 1, :])
                    row = rbc[DH:DH + 1, :]
                    row8 = bass.AP(
                        tensor=row.tensor,
                        offset=row.offset,
                        ap=[list(row.ap[0]), [0, 8]] + [list(d) for d in row.ap[1:]],
                    )
                    nc.sync.dma_start(out=rbc[0:8, :], in_=row8)
                    blk = rbc[0:8, :]
                    blk_rep = bass.AP(
                        tensor=blk.tensor,
                        offset=blk.offset,
                        ap=[list(blk.ap[0]), [0, 7]] + [list(d) for d in blk.ap[1:]],
                    )
                    nc.sync.dma_start(out=rbc[8:DH, :], in_=blk_rep)
                    nc.vector.reciprocal_approx_fast(rbc[0:DH, :], rbc[0:DH, :])
                    if DEBUG_TAPS and pr == 0:
                        nc.sync.dma_start(out=dbg_rbc[hl, ic], in_=rbc)
                    if hl == 0:
                        nc.vector.tensor_mul(
                            ctxn[pr][0:DH, ic * 512:(ic + 1) * 512],
                            cr[(hl, ic)][0:DH, :],
                            rbc[0:DH, :],
                        )
                    else:
                        ch = work.tile([DH, 512], bf16, name=f"ch{hl}_{ic}", tag="ch")
                        nc.vector.tensor_mul(ch, cr[(hl, ic)][0:DH, :], rbc[0:DH, :])
                        nc.sync.dma_start(
                            out=ctxn[pr][DH:2 * DH, ic * 512:(ic + 1) * 512], in_=ch
                        )

        if DEBUG_TAPS:
            for pr in range(NPAIR):
                nc.sync.dma_start(out=dbg_qhT[pr], in_=qhT[pr])
                nc.sync.dma_start(out=dbg_khT[pr], in_=khT[pr])
                nc.sync.dma_start(out=dbg_ctxn[pr], in_=ctxn[pr])
            for jb in range(NJB):
                nc.sync.dma_start(out=dbg_vh[jb], in_=vh_sb[jb])

        # ---- output projection ----
        for ib in range(NIB):
            yp = psum.tile([P, HID], f32, name=f"yp{ib}", tag="mm")
            for pr in range(NPAIR):
                for cc in range(2):
                    nc.tensor.matmul(
                        yp[:, cc * 512:(cc + 1) * 512],
                        lhsT=ctxn[pr][:, ib * P:(ib + 1) * P],
                        rhs=wo_sb[:, pr, cc * 512:(cc + 1) * 512],
                        start=(pr == 0),
                        stop=(pr == NPAIR - 1),
                    )
            y_sb = outp.tile([P, HID], f32, name=f"y{ib}", tag="y")
            nc.scalar.activation(y_sb, yp, AF.Copy)
            nc.sync.dma_start(out=out[ib * P:(ib + 1) * P, :], in_=y_sb)

    nc.compile()
    return nc


def _get_nc():
    global _CACHED_NC
    if _CACHED_NC is None:
        _CACHED_NC = _build_nc()
    return _CACHED_NC


def make_in_maps(q, k, v, attn_bias, Wq, Wk, Wv, Wo, bq, bk, bv, bo):
    scale = DH ** (-0.5)
    in_maps = []
    for core in range(8):
        b, g = divmod(core, 2)
        gs = slice(g * GCOL, (g + 1) * GCOL)
        in_maps.append({
            "qT": np.ascontiguousarray(q[b].T).astype(BF16),
            "kT": np.ascontiguousarray(k[b].T).astype(BF16),
            "vT": np.ascontiguousarray(v[b].T).astype(BF16),
            "wq": (Wq[:, gs] * scale).astype(BF16),
            "wk": np.ascontiguousarray(Wk[:, gs]).astype(BF16),
            "wv": np.ascontiguousarray(Wv[:, gs]).astype(BF16),
            "wo": np.ascontiguousarray(Wo[gs, :]).astype(BF16),
            "bq": (bq[gs] * scale).astype(np.float32),
            "bk": np.ascontiguousarray(bk[gs]).astype(np.float32),
            "bv": np.ascontiguousarray(bv[gs]).astype(BF16),
            "expb": np.exp(
                attn_bias[b, g * 8:(g + 1) * 8].transpose(0, 2, 1)
            ).astype(BF16),
        })
    return in_maps


def kernel(q, k, v, attn_bias, Wq, Wk, Wv, Wo, bq, bk, bv, bo, _trace=False):
    from concourse.bass_utils import run_bass_kernel_spmd

    args = [np.asarray(x, dtype=np.float32) for x in
            (q, k, v, attn_bias, Wq, Wk, Wv, Wo, bq, bk, bv, bo)]
    q, k, v, attn_bias, Wq, Wk, Wv, Wo, bq, bk, bv, bo = args
    nc = _get_nc()
    in_maps = make_in_maps(q, k, v, attn_bias, Wq, Wk, Wv, Wo, bq, bk, bv, bo)
    res = run_bass_kernel_spmd(nc, in_maps, core_ids=list(range(8)), trace=_trace)
    y = np.zeros((4, S, HID), np.float32)
    for core in range(8):
        y[core // 2] += res.results[core]["out"]
    y += bo
    if _trace:
        kernel.last_results = res
    return y
